# revision 29
# baseline (speedup 1.0000x reference)
"""Multi-head causal attention (N=4, L=2048, D=512, H=8) on 8 Trainium2
NeuronCores. Measured ~235 us HW exec, absmax rel err ~7.7e-3 (gate 2e-2).

Sharding: 8 cores = 4 batches x 2 query-tile sets. The attention mask is
causal, so attention for query tile it (128 rows) only covers key tiles
jt <= it. To balance that triangle across the two cores of a batch with a
single SPMD program, core half 0 takes the odd global i-tiles {15,13,...,1}
and half 1 the even ones {14,12,...,0}, both ordered descending. Under that
split the number of local i-tiles whose causal prefix includes key tile jt
is the same on both halves: PROFILE[jt] = ceil((16-jt)/2), so one program
serves both; blocks fully outside the triangle are never computed (72 of
128 (jt,it) blocks per core remain). Each core computes full K/V
projections for its batch, Q projection + causal attention + output
projection for its 8 i-tiles. No collectives.

Engine distribution per core (PE columns are the wall: ~273K at 1 fp16
column/cycle, with the clock at 2.4 GHz for the first ~80 us and then
power-throttled to ~1.2-1.6 GHz):
  PE:   projections, causal score/PV, additive edge-mask matmuls
        (edge-bias block x identity accumulated onto the score PSUM),
        k=65 selector matmuls broadcasting 1/sum to all partitions.
  ACT:  ~60%% of the softmax exp (exact, padding mask folded into the
        per-partition bias) + the VT->SBUF f16 copies (x1/16 so f16 can't
        overflow; the selector matrix carries the 16x back).
  DVE:  the rest of the exp via a one-op f16 Schraudolph
        (uint16(1477.32*x/sqrt(dk) + A*padb + B) bitcast to f16; the
        f32->uint16 convert saturates negatives to +0.0, which zeroes both
        padded keys and -400 edge-masked scores), projection bias-adds,
        denominator reciprocals (bounced through SBUF: the custom-DVE
        reciprocal reads garbage from PSUM), normalize multiplies.
  GpSimd: SBUF-only 1/sum f32->f16 conversions (no PSUM port).

Attention is software-pipelined over flat (head, key-tile, 512-col chunk)
steps: the PE issues step t's score matmul, then the PV matmul of step
t-2, so each step's exp overlaps two steps of PE work and the PE never
stalls (a stall would also drop its p-state). Per-head epilogues split
into column halves: the high half (local tiles 4-7) is final after jt=7
and normalizes mid-head; pair normalizes are deferred 12 steps so their
DVE->GpSimd->PE chain hides behind score/PV work. The output projection
runs i-tiles 4..7 first (they only need the early-finalizing high vtn
columns). Output is staged f16 and upcast on the host.

Host staging (layout/dtype only; all arithmetic happens on device):
  xqT:         [D, LQ] f16   x_q[n].T with i-tile columns in local order
  xkT/xvT:     [D, L] f16    activations pre-transposed
  wqT/.../woT: [D, D] f16    W.T, i.e. [d_in, d_out]
  edgeb:       [128, NJT, 128] f16  additive mask bias 0/-400 for the LAST
               local i-tile of each jt prefix, laid out [i, jt, j] (lhsT of
               the bias-accumulate matmul against an identity rhs); taken
               from the real attention_mask input, so any mask that is
               block-causal at 128 granularity is handled exactly.
  ident:       [128, 128] f16 identity
  padb:        [L] f32       0 / -1e9 log-style padding bias per key
  sel65d:      [65, 128] f16 selector (value 16.0) for the 1/sum broadcast
"""

import numpy as np

import concourse.bass as bass
import concourse.tile as tile
from concourse import bacc, mybir
from concourse.bass_utils import run_bass_kernel_spmd

F32 = mybir.dt.float32
F16 = mybir.dt.float16
U16 = mybir.dt.uint16

N, L, D, H = 4, 2048, 512, 8
DK = D // H          # 64
NCORES = 8
LQ = L // 2          # queries per core
P = 128
DC = D // P          # 4 d-chunks
NJT = L // P         # 16 key tiles
NIT = LQ // P        # 8 query tiles per core

# local i-tile order per core half (descending global tile index)
LOCAL_TILES = [
    [15, 13, 11, 9, 7, 5, 3, 1],   # half 0: odd global i-tiles
    [14, 12, 10, 8, 6, 4, 2, 0],   # half 1: even global i-tiles
]
# number of local i-tiles whose causal prefix includes key tile jt;
# identical for both halves: ceil((16 - jt) / 2)
PROFILE = [-(-(NJT - jt) // 2) for jt in range(NJT)]   # [8,8,7,7,...,1,1]
# local i-tile l accumulates PV over jt = 0 .. JTMAX[l]
JTMAX = [NJT - 1 - 2 * l for l in range(NIT)]          # [15,13,...,1]

# f16 Schraudolph exp: exp(x) ~= bitcast_f16(uint16(1477.3196*x + SCH_B))
SCH_A = 1477.3196
SCH_B = 15316.0
# handicap multiplier on DVE exp cost when balancing the ACT/DVE split
# (DVE also carries bias-adds/normalize work)
DVE_COST_SCALE = 1.6


def build_nc():
    nc = bacc.Bacc("TRN2", target_bir_lowering=False, debug=False,
                   num_devices=NCORES)

    xqT = nc.dram_tensor("xqT", [D, LQ], F16, kind="ExternalInput").ap()
    xkT = nc.dram_tensor("xkT", [D, L], F16, kind="ExternalInput").ap()
    xvT = nc.dram_tensor("xvT", [D, L], F16, kind="ExternalInput").ap()
    wqT = nc.dram_tensor("wqT", [D, D], F16, kind="ExternalInput").ap()
    wkT = nc.dram_tensor("wkT", [D, D], F16, kind="ExternalInput").ap()
    wvT = nc.dram_tensor("wvT", [D, D], F16, kind="ExternalInput").ap()
    woT = nc.dram_tensor("woT", [D, D], F16, kind="ExternalInput").ap()
    bq = nc.dram_tensor("bq", [D], F32, kind="ExternalInput").ap()
    bk = nc.dram_tensor("bk", [D], F32, kind="ExternalInput").ap()
    bv = nc.dram_tensor("bv", [D], F32, kind="ExternalInput").ap()
    bo = nc.dram_tensor("bo", [D], F32, kind="ExternalInput").ap()
    sel65d = nc.dram_tensor("sel65d", [DK + 1, P], F16, kind="ExternalInput").ap()
    edgeb = nc.dram_tensor("edgeb", [P, NJT, P], F16, kind="ExternalInput").ap()
    identd = nc.dram_tensor("identd", [P, P], F16, kind="ExternalInput").ap()
    padb = nc.dram_tensor("padb", [L], F32, kind="ExternalInput").ap()
    out = nc.dram_tensor("out", [LQ, D], F16, kind="ExternalOutput").ap()

    with tile.TileContext(nc) as tc, nc.allow_low_precision(
            reason="f16 matmul operands; accumulation stays f32"):
        build_kernel(tc, xqT, xkT, xvT, wqT, wkT, wvT, woT,
                     bq, bk, bv, bo, sel65d, edgeb, identd, padb, out)
    nc.compile()
    return nc


def build_kernel(tc, xqT, xkT, xvT, wqT, wkT, wvT, woT,
                 bq, bk, bv, bo, sel65d, edgeb, identd, padb, out):
    nc = tc.nc
    Exp = mybir.ActivationFunctionType.Exp
    Copy = mybir.ActivationFunctionType.Copy
    Mult = mybir.AluOpType.mult
    Add = mybir.AluOpType.add

    with (
        tc.tile_pool(name="persist", bufs=1) as persist,
        tc.tile_pool(name="bigpersist", bufs=1) as bigpersist,
    ):
        # ---- persistent tiles --------------------------------------------
        qt_sb = bigpersist.tile([P, DC, LQ], F16, tag="qt")
        kt_sb = bigpersist.tile([P, DC, L], F16, tag="kt")
        # V natural [j, d], fp16, heads interleaved with a ones column after
        # each head's 64 dims: [j-tile, head, 65]
        v_sb = bigpersist.tile([P, NJT, H, DK + 1], F16, tag="v")
        nc.vector.memset(v_sb[:, :, :, DK:DK + 1], 1.0)
        wo_sb = persist.tile([P, DC, D], F16, tag="wo")
        bo_bc = persist.tile([P, D], F32, tag="bo")
        sel65 = persist.tile([DK + 1, P], F16, tag="sel65")
        edgeb_sb = persist.tile([P, NJT, P], F16, tag="edgeb")
        ident_sb = persist.tile([P, P], F16, tag="ident")
        padb_sb = persist.tile([P, NJT], F32, tag="padb")
        padsch_sb = persist.tile([P, NJT], F32, tag="padsch")
        # denominator rows parked at partitions 0/64, one slot per head
        # pair (reciprocal_approx_fast is a custom DVE op: PSUM inputs give
        # garbage on HW, so the rows are bounced through SBUF first); rows
        # 1..63 stay 1.0 so the reciprocal and the f16 convert stay finite
        den65 = persist.tile([DK + 1, H // 2, LQ], F32, tag="den65")
        nc.vector.memset(den65, 1.0)
        rsf32 = persist.tile([DK + 1, H // 2, LQ], F32, tag="rsf32")
        rsh65 = persist.tile([DK + 1, H // 2, LQ], F16, tag="rsh65")

        # ---- projections --------------------------------------------------
        with (
            tc.tile_pool(name="wproj", bufs=1) as wproj,
            tc.tile_pool(name="xstage", bufs=3) as xstage,
            tc.tile_pool(name="projps", bufs=4, space="PSUM") as projps,
        ):
            wq_sb = wproj.tile([P, DC, D], F16, tag="wq")
            nc.sync.dma_start(out=wq_sb, in_=wqT.rearrange("(c p) n -> p c n", p=P))
            wk_sb = wproj.tile([P, DC, D], F16, tag="wk")
            nc.sync.dma_start(out=wk_sb, in_=wkT.rearrange("(c p) n -> p c n", p=P))
            wv_sb = wproj.tile([P, DC, D], F16, tag="wv")
            nc.sync.dma_start(out=wv_sb, in_=wvT.rearrange("(c p) n -> p c n", p=P))
            bq_col = wproj.tile([P, DC], F32, tag="bqc")
            nc.sync.dma_start(out=bq_col, in_=bq.rearrange("(c p) -> p c", p=P))
            bk_col = wproj.tile([P, DC], F32, tag="bkc")
            nc.sync.dma_start(out=bk_col, in_=bk.rearrange("(c p) -> p c", p=P))
            bv_bc = wproj.tile([P, D], F32, tag="bvbc")
            nc.sync.dma_start(
                out=bv_bc,
                in_=bass.AP(tensor=bv.tensor, offset=bv.offset,
                            ap=[[0, P], [1, D]]))

            # Q projection first (all scores need it), then K and V
            # interleaved per j-block so attention can start early.
            def qk_proj(w_sb, b_col, out_sb, xT, jb):
                xt = xstage.tile([P, DC, 512], F16, tag="xstage")
                xre = xT.rearrange("(c p) m -> p c m", p=P)
                for k in range(DC):
                    nc.sync.dma_start(
                        out=xt[:, k, :],
                        in_=xre[:, k, jb * 512:(jb + 1) * 512])
                for c in range(DC):
                    ps = projps.tile([P, 512], F32, tag="projps")
                    for k in range(DC):
                        nc.tensor.matmul(
                            ps, lhsT=w_sb[:, k, c * P:(c + 1) * P],
                            rhs=xt[:, k, :],
                            start=(k == 0), stop=(k == DC - 1))
                    nc.vector.tensor_scalar_add(
                        out=out_sb[:, c, jb * 512:(jb + 1) * 512],
                        in0=ps, scalar1=b_col[:, c:c + 1])

            def v_proj(jb):
                xt = xstage.tile([P, DC, 512], F16, tag="xstage")
                xre = xvT.rearrange("(c p) m -> p c m", p=P)
                for k in range(DC):
                    nc.sync.dma_start(
                        out=xt[:, k, :],
                        in_=xre[:, k, jb * 512:(jb + 1) * 512])
                for jtl in range(4):
                    jt = jb * 4 + jtl
                    ps = projps.tile([P, D], F32, tag="projpsv")
                    for k in range(DC):
                        nc.tensor.matmul(
                            ps, lhsT=xt[:, k, jtl * P:(jtl + 1) * P],
                            rhs=wv_sb[:, k, :],
                            start=(k == 0), stop=(k == DC - 1))
                    nc.vector.tensor_add(
                        out=v_sb[:, jt, :, 0:DK],
                        in0=ps.rearrange("p (h d) -> p h d", h=H),
                        in1=bv_bc.rearrange("p (h d) -> p h d", h=H))

            for jb in range(LQ // 512):
                qk_proj(wq_sb, bq_col, qt_sb, xqT, jb)
                if jb == 0:
                    # attention prerequisites, queued behind the critical
                    # first-projection DMAs
                    nc.sync.dma_start(
                        out=padb_sb,
                        in_=padb.rearrange("(t p) -> p t", p=P))
                    nc.vector.tensor_scalar(
                        out=padsch_sb, in0=padb_sb, scalar1=SCH_A,
                        scalar2=SCH_B, op0=Mult, op1=Add)
                    nc.sync.dma_start(out=edgeb_sb, in_=edgeb)
                    nc.sync.dma_start(out=ident_sb, in_=identd)
                    nc.sync.dma_start(out=sel65, in_=sel65d)
            for jb in range(L // 512):
                qk_proj(wk_sb, bk_col, kt_sb, xkT, jb)
                v_proj(jb)
            nc.sync.dma_start(out=wo_sb,
                              in_=woT.rearrange("(c p) n -> p c n", p=P))
            nc.sync.dma_start(
                out=bo_bc,
                in_=bass.AP(tensor=bo.tensor, offset=bo.offset,
                            ap=[[0, P], [1, D]]))

        # ---- attention ----------------------------------------------------
        with (
            tc.tile_pool(name="stps", bufs=3, space="PSUM") as stps,
            tc.tile_pool(name="vtps", bufs=2, space="PSUM") as vtps,
            tc.tile_pool(name="ppool", bufs=4) as ppool,
            tc.tile_pool(name="upool", bufs=4) as upool,
        ):
            vtn_sb = bigpersist.tile([P, DC, LQ], F16, tag="vtn")

            # flat steps: (h, jt, c0, w, edge_local or None)
            steps = []
            for h in range(H):
                for jt in range(NJT):
                    cols = PROFILE[jt] * P
                    el = cols - P
                    for c0 in range(0, cols, 512):
                        w = min(512, cols - c0)
                        e = el - c0 if c0 <= el < c0 + 512 else None
                        steps.append((h, jt, c0, w, e))

            # greedy cost-balanced ACT/DVE assignment for the exp
            acc_a = acc_d = 0.0
            exp_eng = []
            for h, jt, c0, w, e in steps:
                ca = (w + 440) / 1.2
                cd = (w + 240) / 0.96 * DVE_COST_SCALE
                if acc_a + ca <= acc_d + cd:
                    exp_eng.append("A")
                    acc_a += ca
                else:
                    exp_eng.append("D")
                    acc_d += cd

            def issue_score(idx):
                """score (+ edge-bias) matmuls and exp for step idx; the
                exp runs on ACT (exact) or DVE (Schraudolph) per exp_eng."""
                h, jt, c0, w, e = steps[idx]
                hc, ho = h // 2, (h % 2) * DK
                st = stps.tile([P, 512], F32, tag="st")
                nc.tensor.matmul(
                    st[:, 0:w],
                    lhsT=kt_sb[ho:ho + DK, hc, jt * P:(jt + 1) * P],
                    rhs=qt_sb[ho:ho + DK, hc, c0:c0 + w],
                    start=True, stop=(e is None))
                if e is not None:
                    nc.tensor.matmul(
                        st[:, e:e + P], lhsT=edgeb_sb[:, jt, :],
                        rhs=ident_sb, start=False, stop=True)
                if exp_eng[idx] == "A":
                    pe = ppool.tile([P, 512], F16, tag="pe")
                    nc.scalar.activation(out=pe[:, 0:w], in_=st[:, 0:w],
                                         func=Exp, scale=1.0 / np.sqrt(DK),
                                         bias=padb_sb[:, jt:jt + 1])
                    return pe
                pu = upool.tile([P, 512], U16, tag="pu")
                nc.vector.tensor_scalar(
                    out=pu[:, 0:w], in0=st[:, 0:w],
                    scalar1=SCH_A / np.sqrt(DK),
                    scalar2=padsch_sb[:, jt:jt + 1],
                    op0=Mult, op1=Add)
                return pu.bitcast(F16)

            def issue_pv(idx, vt, pe):
                h, jt, c0, w, e = steps[idx]
                nc.tensor.matmul(
                    vt[:, c0:c0 + w],
                    lhsT=v_sb[:, jt, h, :],
                    rhs=pe[:, 0:w],
                    start=(jt == 0), stop=(jt == JTMAX[c0 // P]),
                    skip_group_check=False)

            def issue_tail(h, vt, half):
                """per-head epilogue for one 512-col half: stash the
                unnormalized VT (scaled by 1/16 so f16 cannot overflow;
                sel65 carries the 16x back) and park the denominator row.
                The high half (cols 512:1024, local tiles 4-7) is final
                after jt=7 so it runs mid-head."""
                hc, ho = h // 2, (h % 2) * DK
                s = slice(half * 512, half * 512 + 512)
                nc.scalar.activation(out=vtn_sb[ho:ho + DK, hc, s],
                                     in_=vt[0:DK, s], func=Copy,
                                     scale=1.0 / 16.0)
                nc.vector.tensor_copy(out=den65[ho:ho + 1, hc, s],
                                      in_=vt[DK:DK + 1, s])

            def issue_tail2(p, half):
                """pair normalize for one 512-col half, issued a few steps
                later so the serial DVE->GpSimd->PE->DVE chain hides behind
                score/PV work."""
                s = slice(half * 512, half * 512 + 512)
                nc.vector.reciprocal_approx_fast(out=rsf32[:, p, s],
                                                 in_=den65[:, p, s])
                nc.gpsimd.tensor_copy(out=rsh65[:, p, s], in_=rsf32[:, p, s])
                rbp = stps.tile([P, 512], F32, name="rbp", tag="st")
                nc.tensor.matmul(rbp, lhsT=sel65, rhs=rsh65[:, p, s],
                                 start=True, stop=True)
                nc.vector.tensor_mul(vtn_sb[:, p, s], vtn_sb[:, p, s], rbp)

            # software pipeline, depth 2; tail halves issued as soon as
            # their columns are final, pair normalizes deferred
            LAG = 2
            TAIL2_DELAY = 12
            vts = {}
            pes = {}
            tail2_at = {}

            def after_pv(j, at_idx):
                jh, jjt, jc0, jw, je = steps[j]
                if jjt == 7 and jc0 == 512:
                    issue_tail(jh, vts[jh], 1)
                    if jh % 2 == 1:
                        tail2_at[at_idx + TAIL2_DELAY] = (jh // 2, 1)
                elif jjt == NJT - 1:
                    issue_tail(jh, vts[jh], 0)
                    if jh % 2 == 1:
                        tail2_at[at_idx + TAIL2_DELAY] = (jh // 2, 0)

            for idx, (h, jt, c0, w, e) in enumerate(steps):
                if jt == 0 and c0 == 0:
                    vts[h] = vtps.tile([DK + 1, LQ], F32, name="vt", tag="vt")
                pes[idx] = issue_score(idx)
                j = idx - LAG
                if j >= 0:
                    issue_pv(j, vts[steps[j][0]], pes.pop(j))
                    after_pv(j, idx)
                t2 = tail2_at.pop(idx, None)
                if t2 is not None:
                    issue_tail2(*t2)
            for j in range(len(steps) - LAG, len(steps)):
                issue_pv(j, vts[steps[j][0]], pes.pop(j))
                after_pv(j, len(steps))
            for _, t2 in sorted(tail2_at.items()):
                issue_tail2(*t2)

            # ---- output projection (reuses PSUM slots so it can overlap
            # the last head pair's tail) ----
            # i-tiles 4..7 first: they read only the high vtn columns,
            # whose normalize finished mid-head-7, so they overlap the
            # last pair's low-half tail chain
            with tc.tile_pool(name="obuf", bufs=3) as obuf:
                for it in list(range(4, NIT)) + list(range(4)):
                    po = stps.tile([P, D], F32, name="po", tag="st")
                    for c in range(DC):
                        nc.tensor.matmul(
                            po, lhsT=vtn_sb[:, c, it * P:(it + 1) * P],
                            rhs=wo_sb[:, c, :], start=(c == 0),
                            stop=(c == DC - 1))
                    ob = obuf.tile([P, D], F16, tag="ob")
                    nc.vector.tensor_add(ob, po, bo_bc)
                    nc.sync.dma_start(out=out[it * P:(it + 1) * P, :], in_=ob)


_NC_CACHE = None


def _get_nc():
    global _NC_CACHE
    if _NC_CACHE is None:
        _NC_CACHE = build_nc()
    return _NC_CACHE


def _sel65_const():
    # 16x compensates the 1/16 scale on the unnormalized-VT f16 stash
    sel = np.zeros((DK + 1, P), dtype=np.float16)
    sel[0, 0:DK] = 16.0
    sel[DK, DK:P] = 16.0
    return sel


def make_in_maps(x_q, x_k, x_v, padding_mask, attention_mask,
                 Wq, bq, Wk, bk, Wv, bv, Wo, bo):
    f16, f32 = np.float16, np.float32
    shared = {
        "wqT": np.ascontiguousarray(np.asarray(Wq, dtype=f32).T).astype(f16),
        "wkT": np.ascontiguousarray(np.asarray(Wk, dtype=f32).T).astype(f16),
        "wvT": np.ascontiguousarray(np.asarray(Wv, dtype=f32).T).astype(f16),
        "woT": np.ascontiguousarray(np.asarray(Wo, dtype=f32).T).astype(f16),
        "bq": np.asarray(bq, dtype=f32), "bk": np.asarray(bk, dtype=f32),
        "bv": np.asarray(bv, dtype=f32), "bo": np.asarray(bo, dtype=f32),
        "sel65d": _sel65_const(),
        "identd": np.eye(P, dtype=f16),
    }
    am = np.asarray(attention_mask, dtype=f32)
    # additive edge-mask bias [i, jt, j] (0 keep / -400 drop) for the
    # last local i-tile of each jt prefix
    edge_half = []
    for half in range(2):
        loc = LOCAL_TILES[half]
        e = np.empty((NJT, P, P), dtype=f16)       # [jt, i, j]
        for jt in range(NJT):
            g = loc[PROFILE[jt] - 1]
            blk = am[g * P:(g + 1) * P, jt * P:(jt + 1) * P]
            e[jt] = ((blk - 1.0) * 400.0).astype(f16)
        edge_half.append(np.ascontiguousarray(e.transpose(1, 0, 2)))
    xT = [np.asarray(x, dtype=f32).transpose(0, 2, 1).astype(f16)
          for x in (x_q, x_k, x_v)]
    in_maps = []
    for core in range(NCORES):
        n, half = divmod(core, 2)
        loc = LOCAL_TILES[half]
        xq_loc = xT[0][n].reshape(D, NJT, P)[:, loc, :].reshape(D, LQ)
        padb = (np.asarray(padding_mask[n], dtype=f32) - 1.0) * 1e9
        in_maps.append(dict(
            shared,
            xqT=np.ascontiguousarray(xq_loc),
            xkT=np.ascontiguousarray(xT[1][n]),
            xvT=np.ascontiguousarray(xT[2][n]),
            edgeb=edge_half[half],
            padb=padb,
        ))
    return in_maps


def gather_out(results):
    full = np.empty((N, L, D), dtype=np.float32)
    for core in range(NCORES):
        n, half = divmod(core, 2)
        o = results[core]["out"]
        for l, g in enumerate(LOCAL_TILES[half]):
            full[n, g * P:(g + 1) * P, :] = o[l * P:(l + 1) * P, :]
    return full


def kernel(x_q, x_k, x_v, padding_mask, attention_mask,
           Wq, bq, Wk, bk, Wv, bv, Wo, bo):
    nc = _get_nc()
    in_maps = make_in_maps(x_q, x_k, x_v, padding_mask, attention_mask,
                           Wq, bq, Wk, bk, Wv, bv, Wo, bo)
    res = run_bass_kernel_spmd(nc, in_maps, core_ids=list(range(NCORES)))
    return gather_out(res.results)


# revision 30
# speedup vs baseline: 1.0493x; 1.0493x over previous
"""Multi-head causal attention (N=4, L=2048, D=512, H=8) on 8 Trainium2
NeuronCores. Measured ~235 us HW exec, absmax rel err ~7.7e-3 (gate 2e-2).

Sharding: 8 cores = 4 batches x 2 query-tile sets. The attention mask is
causal, so attention for query tile it (128 rows) only covers key tiles
jt <= it. To balance that triangle across the two cores of a batch with a
single SPMD program, core half 0 takes the odd global i-tiles {15,13,...,1}
and half 1 the even ones {14,12,...,0}, both ordered descending. Under that
split the number of local i-tiles whose causal prefix includes key tile jt
is the same on both halves: PROFILE[jt] = ceil((16-jt)/2), so one program
serves both; blocks fully outside the triangle are never computed (72 of
128 (jt,it) blocks per core remain). Each core computes full K/V
projections for its batch, Q projection + causal attention + output
projection for its 8 i-tiles. No collectives.

Engine distribution per core (PE columns are the wall: ~273K at 1 fp16
column/cycle, with the clock at 2.4 GHz for the first ~80 us and then
power-throttled to ~1.2-1.6 GHz):
  PE:   projections, causal score/PV, additive edge-mask matmuls
        (edge-bias block x identity accumulated onto the score PSUM),
        k=65 selector matmuls broadcasting 1/sum to all partitions.
  ACT:  ~60%% of the softmax exp (exact, padding mask folded into the
        per-partition bias) + the VT->SBUF f16 copies (x1/16 so f16 can't
        overflow; the selector matrix carries the 16x back).
  DVE:  the rest of the exp via a one-op f16 Schraudolph
        (uint16(1477.32*x/sqrt(dk) + A*padb + B) bitcast to f16; the
        f32->uint16 convert saturates negatives to +0.0, which zeroes both
        padded keys and -400 edge-masked scores), projection bias-adds,
        denominator reciprocals (bounced through SBUF: the custom-DVE
        reciprocal reads garbage from PSUM), normalize multiplies.
  GpSimd: SBUF-only 1/sum f32->f16 conversions (no PSUM port).

Attention is software-pipelined over flat (head, key-tile, 512-col chunk)
steps: the PE issues step t's score matmul, then the PV matmul of step
t-2, so each step's exp overlaps two steps of PE work and the PE never
stalls (a stall would also drop its p-state). Per-head epilogues split
into column halves: the high half (local tiles 4-7) is final after jt=7
and normalizes mid-head; pair normalizes are deferred 12 steps so their
DVE->GpSimd->PE chain hides behind score/PV work. The output projection
runs i-tiles 4..7 first (they only need the early-finalizing high vtn
columns). Output is staged f16 and upcast on the host.

Host staging (layout/dtype only; all arithmetic happens on device):
  xqT:         [D, LQ] f16   x_q[n].T with i-tile columns in local order
  xkT/xvT:     [D, L] f16    activations pre-transposed
  wqT/.../woT: [D, D] f16    W.T, i.e. [d_in, d_out]
  edgeb:       [128, NJT, 128] f16  additive mask bias 0/-400 for the LAST
               local i-tile of each jt prefix, laid out [i, jt, j] (lhsT of
               the bias-accumulate matmul against an identity rhs); taken
               from the real attention_mask input, so any mask that is
               block-causal at 128 granularity is handled exactly.
  ident:       [128, 128] f16 identity
  padb:        [L] f32       0 / -1e9 log-style padding bias per key
  sel65d:      [65, 128] f16 selector (value 16.0) for the 1/sum broadcast
"""

import numpy as np

import concourse.bass as bass
import concourse.tile as tile
from concourse import bacc, mybir
from concourse.bass_utils import run_bass_kernel_spmd

F32 = mybir.dt.float32
F16 = mybir.dt.float16
U16 = mybir.dt.uint16

N, L, D, H = 4, 2048, 512, 8
DK = D // H          # 64
NCORES = 8
LQ = L // 2          # queries per core
P = 128
DC = D // P          # 4 d-chunks
NJT = L // P         # 16 key tiles
NIT = LQ // P        # 8 query tiles per core

# local i-tile order per core half (descending global tile index)
LOCAL_TILES = [
    [15, 13, 11, 9, 7, 5, 3, 1],   # half 0: odd global i-tiles
    [14, 12, 10, 8, 6, 4, 2, 0],   # half 1: even global i-tiles
]
# number of local i-tiles whose causal prefix includes key tile jt;
# identical for both halves: ceil((16 - jt) / 2)
PROFILE = [-(-(NJT - jt) // 2) for jt in range(NJT)]   # [8,8,7,7,...,1,1]
# local i-tile l accumulates PV over jt = 0 .. JTMAX[l]
JTMAX = [NJT - 1 - 2 * l for l in range(NIT)]          # [15,13,...,1]

# f16 Schraudolph exp: exp(x) ~= bitcast_f16(uint16(1477.3196*x + SCH_B))
SCH_A = 1477.3196
SCH_B = 15316.0
# handicap multiplier on DVE exp cost when balancing the ACT/DVE split
# (DVE also carries bias-adds/normalize work)
DVE_COST_SCALE = 1.6


def build_nc():
    nc = bacc.Bacc("TRN2", target_bir_lowering=False, debug=False,
                   num_devices=NCORES)

    xqT = nc.dram_tensor("xqT", [D, LQ], F16, kind="ExternalInput").ap()
    xkT = nc.dram_tensor("xkT", [D, L], F16, kind="ExternalInput").ap()
    xvT = nc.dram_tensor("xvT", [D, L], F16, kind="ExternalInput").ap()
    wqT = nc.dram_tensor("wqT", [D, D], F16, kind="ExternalInput").ap()
    wkT = nc.dram_tensor("wkT", [D, D], F16, kind="ExternalInput").ap()
    wvT = nc.dram_tensor("wvT", [D, D], F16, kind="ExternalInput").ap()
    woT = nc.dram_tensor("woT", [D, D], F16, kind="ExternalInput").ap()
    bq = nc.dram_tensor("bq", [D], F32, kind="ExternalInput").ap()
    bk = nc.dram_tensor("bk", [D], F32, kind="ExternalInput").ap()
    bv = nc.dram_tensor("bv", [D], F32, kind="ExternalInput").ap()
    bo = nc.dram_tensor("bo", [D], F32, kind="ExternalInput").ap()
    sel65d = nc.dram_tensor("sel65d", [DK + 1, P], F16, kind="ExternalInput").ap()
    edgeb = nc.dram_tensor("edgeb", [P, NJT, P], F16, kind="ExternalInput").ap()
    identd = nc.dram_tensor("identd", [P, P], F16, kind="ExternalInput").ap()
    padb = nc.dram_tensor("padb", [L], F32, kind="ExternalInput").ap()
    out = nc.dram_tensor("out", [LQ, D], F16, kind="ExternalOutput").ap()

    with tile.TileContext(nc) as tc, nc.allow_low_precision(
            reason="f16 matmul operands; accumulation stays f32"):
        build_kernel(tc, xqT, xkT, xvT, wqT, wkT, wvT, woT,
                     bq, bk, bv, bo, sel65d, edgeb, identd, padb, out)
    nc.compile()
    return nc


def build_kernel(tc, xqT, xkT, xvT, wqT, wkT, wvT, woT,
                 bq, bk, bv, bo, sel65d, edgeb, identd, padb, out):
    nc = tc.nc
    Exp = mybir.ActivationFunctionType.Exp
    Copy = mybir.ActivationFunctionType.Copy
    Mult = mybir.AluOpType.mult
    Add = mybir.AluOpType.add

    with (
        tc.tile_pool(name="persist", bufs=1) as persist,
        tc.tile_pool(name="bigpersist", bufs=1) as bigpersist,
    ):
        # ---- persistent tiles --------------------------------------------
        qt_sb = bigpersist.tile([P, DC, LQ], F16, tag="qt")
        kt_sb = bigpersist.tile([P, DC, L], F16, tag="kt")
        # V natural [j, d], fp16, heads interleaved with a ones column after
        # each head's 64 dims: [j-tile, head, 65]
        v_sb = bigpersist.tile([P, NJT, H, DK + 1], F16, tag="v")
        nc.vector.memset(v_sb[:, :, :, DK:DK + 1], 1.0)
        wo_sb = persist.tile([P, DC, D], F16, tag="wo")
        bo_bc = persist.tile([P, D], F32, tag="bo")
        sel65 = persist.tile([DK + 1, P], F16, tag="sel65")
        edgeb_sb = persist.tile([P, NJT, P], F16, tag="edgeb")
        ident_sb = persist.tile([P, P], F16, tag="ident")
        padb_sb = persist.tile([P, NJT], F32, tag="padb")
        padsch_sb = persist.tile([P, NJT], F32, tag="padsch")
        # denominator rows parked at partitions 0/64, one slot per head
        # pair (reciprocal_approx_fast is a custom DVE op: PSUM inputs give
        # garbage on HW, so the rows are bounced through SBUF first); rows
        # 1..63 stay 1.0 so the reciprocal and the f16 convert stay finite
        den65 = persist.tile([DK + 1, H // 2, LQ], F32, tag="den65")
        nc.vector.memset(den65, 1.0)
        rsf32 = persist.tile([DK + 1, H // 2, LQ], F32, tag="rsf32")
        rsh65 = persist.tile([DK + 1, H // 2, LQ], F16, tag="rsh65")

        # ---- projections --------------------------------------------------
        with (
            tc.tile_pool(name="wproj", bufs=1) as wproj,
            tc.tile_pool(name="xstage", bufs=3) as xstage,
            tc.tile_pool(name="projps", bufs=4, space="PSUM") as projps,
        ):
            # only the Q-projection operands dispatch ahead of the first
            # matmul; everything else queues behind the first x block
            wq_sb = wproj.tile([P, DC, D], F16, tag="wq")
            nc.sync.dma_start(out=wq_sb, in_=wqT.rearrange("(c p) n -> p c n", p=P))
            bq_col = wproj.tile([P, DC], F32, tag="bqc")
            nc.sync.dma_start(out=bq_col, in_=bq.rearrange("(c p) -> p c", p=P))
            wk_sb = wproj.tile([P, DC, D], F16, tag="wk")
            wv_sb = wproj.tile([P, DC, D], F16, tag="wv")
            bk_col = wproj.tile([P, DC], F32, tag="bkc")
            bv_bc = wproj.tile([P, D], F32, tag="bvbc")

            # Q projection first (all scores need it), then K and V
            # interleaved per j-block so attention can start early.
            def qk_proj(w_sb, b_col, out_sb, xT, jb):
                xt = xstage.tile([P, DC, 512], F16, tag="xstage")
                xre = xT.rearrange("(c p) m -> p c m", p=P)
                nc.sync.dma_start(
                    out=xt, in_=xre[:, :, jb * 512:(jb + 1) * 512])
                for c in range(DC):
                    ps = projps.tile([P, 512], F32, tag="projps")
                    for k in range(DC):
                        nc.tensor.matmul(
                            ps, lhsT=w_sb[:, k, c * P:(c + 1) * P],
                            rhs=xt[:, k, :],
                            start=(k == 0), stop=(k == DC - 1))
                    nc.vector.tensor_scalar_add(
                        out=out_sb[:, c, jb * 512:(jb + 1) * 512],
                        in0=ps, scalar1=b_col[:, c:c + 1])

            def v_proj(jb):
                xt = xstage.tile([P, DC, 512], F16, tag="xstage")
                xre = xvT.rearrange("(c p) m -> p c m", p=P)
                nc.sync.dma_start(
                    out=xt, in_=xre[:, :, jb * 512:(jb + 1) * 512])
                for jtl in range(4):
                    jt = jb * 4 + jtl
                    ps = projps.tile([P, D], F32, tag="projpsv")
                    for k in range(DC):
                        nc.tensor.matmul(
                            ps, lhsT=xt[:, k, jtl * P:(jtl + 1) * P],
                            rhs=wv_sb[:, k, :],
                            start=(k == 0), stop=(k == DC - 1))
                    nc.vector.tensor_add(
                        out=v_sb[:, jt, :, 0:DK],
                        in0=ps.rearrange("p (h d) -> p h d", h=H),
                        in1=bv_bc.rearrange("p (h d) -> p h d", h=H))

            for jb in range(LQ // 512):
                qk_proj(wq_sb, bq_col, qt_sb, xqT, jb)
                if jb == 0:
                    # remaining operands + attention prerequisites, queued
                    # behind the critical first-projection DMAs
                    nc.sync.dma_start(
                        out=wk_sb,
                        in_=wkT.rearrange("(c p) n -> p c n", p=P))
                    nc.sync.dma_start(
                        out=wv_sb,
                        in_=wvT.rearrange("(c p) n -> p c n", p=P))
                    nc.sync.dma_start(out=bk_col,
                                      in_=bk.rearrange("(c p) -> p c", p=P))
                    nc.sync.dma_start(
                        out=bv_bc,
                        in_=bass.AP(tensor=bv.tensor, offset=bv.offset,
                                    ap=[[0, P], [1, D]]))
                    nc.sync.dma_start(
                        out=padb_sb,
                        in_=padb.rearrange("(t p) -> p t", p=P))
                    nc.vector.tensor_scalar(
                        out=padsch_sb, in0=padb_sb, scalar1=SCH_A,
                        scalar2=SCH_B, op0=Mult, op1=Add)
                    nc.sync.dma_start(out=edgeb_sb, in_=edgeb)
                    nc.sync.dma_start(out=ident_sb, in_=identd)
                    nc.sync.dma_start(out=sel65, in_=sel65d)
            for jb in range(L // 512):
                qk_proj(wk_sb, bk_col, kt_sb, xkT, jb)
                v_proj(jb)
            nc.sync.dma_start(out=wo_sb,
                              in_=woT.rearrange("(c p) n -> p c n", p=P))
            nc.sync.dma_start(
                out=bo_bc,
                in_=bass.AP(tensor=bo.tensor, offset=bo.offset,
                            ap=[[0, P], [1, D]]))

        # ---- attention ----------------------------------------------------
        with (
            tc.tile_pool(name="stps", bufs=3, space="PSUM") as stps,
            tc.tile_pool(name="vtps", bufs=2, space="PSUM") as vtps,
            tc.tile_pool(name="ppool", bufs=4) as ppool,
            tc.tile_pool(name="upool", bufs=4) as upool,
        ):
            vtn_sb = bigpersist.tile([P, DC, LQ], F16, tag="vtn")

            # flat steps: (h, jt, c0, w, edge_local or None)
            steps = []
            for h in range(H):
                for jt in range(NJT):
                    cols = PROFILE[jt] * P
                    el = cols - P
                    for c0 in range(0, cols, 512):
                        w = min(512, cols - c0)
                        e = el - c0 if c0 <= el < c0 + 512 else None
                        steps.append((h, jt, c0, w, e))

            # greedy cost-balanced ACT/DVE assignment for the exp
            acc_a = acc_d = 0.0
            exp_eng = []
            for h, jt, c0, w, e in steps:
                ca = (w + 440) / 1.2
                cd = (w + 240) / 0.96 * DVE_COST_SCALE
                if acc_a + ca <= acc_d + cd:
                    exp_eng.append("A")
                    acc_a += ca
                else:
                    exp_eng.append("D")
                    acc_d += cd

            def issue_score(idx):
                """score (+ edge-bias) matmuls and exp for step idx; the
                exp runs on ACT (exact) or DVE (Schraudolph) per exp_eng."""
                h, jt, c0, w, e = steps[idx]
                hc, ho = h // 2, (h % 2) * DK
                st = stps.tile([P, 512], F32, tag="st")
                nc.tensor.matmul(
                    st[:, 0:w],
                    lhsT=kt_sb[ho:ho + DK, hc, jt * P:(jt + 1) * P],
                    rhs=qt_sb[ho:ho + DK, hc, c0:c0 + w],
                    start=True, stop=(e is None))
                if e is not None:
                    nc.tensor.matmul(
                        st[:, e:e + P], lhsT=edgeb_sb[:, jt, :],
                        rhs=ident_sb, start=False, stop=True)
                if exp_eng[idx] == "A":
                    pe = ppool.tile([P, 512], F16, tag="pe")
                    nc.scalar.activation(out=pe[:, 0:w], in_=st[:, 0:w],
                                         func=Exp, scale=1.0 / np.sqrt(DK),
                                         bias=padb_sb[:, jt:jt + 1])
                    return pe
                pu = upool.tile([P, 512], U16, tag="pu")
                nc.vector.tensor_scalar(
                    out=pu[:, 0:w], in0=st[:, 0:w],
                    scalar1=SCH_A / np.sqrt(DK),
                    scalar2=padsch_sb[:, jt:jt + 1],
                    op0=Mult, op1=Add)
                return pu.bitcast(F16)

            def issue_pv(idx, vt, pe):
                h, jt, c0, w, e = steps[idx]
                nc.tensor.matmul(
                    vt[:, c0:c0 + w],
                    lhsT=v_sb[:, jt, h, :],
                    rhs=pe[:, 0:w],
                    start=(jt == 0), stop=(jt == JTMAX[c0 // P]),
                    skip_group_check=False)

            def issue_tail(h, vt, half):
                """per-head epilogue for one 512-col half: stash the
                unnormalized VT (scaled by 1/16 so f16 cannot overflow;
                sel65 carries the 16x back) and park the denominator row.
                The high half (cols 512:1024, local tiles 4-7) is final
                after jt=7 so it runs mid-head."""
                hc, ho = h // 2, (h % 2) * DK
                s = slice(half * 512, half * 512 + 512)
                nc.scalar.activation(out=vtn_sb[ho:ho + DK, hc, s],
                                     in_=vt[0:DK, s], func=Copy,
                                     scale=1.0 / 16.0)
                nc.vector.tensor_copy(out=den65[ho:ho + 1, hc, s],
                                      in_=vt[DK:DK + 1, s])

            def issue_tail2(p, half):
                """pair normalize for one 512-col half, issued a few steps
                later so the serial DVE->GpSimd->PE->DVE chain hides behind
                score/PV work."""
                s = slice(half * 512, half * 512 + 512)
                nc.vector.reciprocal_approx_fast(out=rsf32[:, p, s],
                                                 in_=den65[:, p, s])
                nc.gpsimd.tensor_copy(out=rsh65[:, p, s], in_=rsf32[:, p, s])
                rbp = stps.tile([P, 512], F32, name="rbp", tag="st")
                nc.tensor.matmul(rbp, lhsT=sel65, rhs=rsh65[:, p, s],
                                 start=True, stop=True)
                nc.vector.tensor_mul(vtn_sb[:, p, s], vtn_sb[:, p, s], rbp)

            # software pipeline, depth 2; tail halves issued as soon as
            # their columns are final, pair normalizes deferred
            LAG = 2
            TAIL2_DELAY = 12
            vts = {}
            pes = {}
            tail2_at = {}

            def after_pv(j, at_idx):
                jh, jjt, jc0, jw, je = steps[j]
                if jjt == 7 and jc0 == 512:
                    issue_tail(jh, vts[jh], 1)
                    if jh % 2 == 1:
                        tail2_at[at_idx + TAIL2_DELAY] = (jh // 2, 1)
                elif jjt == NJT - 1:
                    issue_tail(jh, vts[jh], 0)
                    if jh % 2 == 1:
                        tail2_at[at_idx + TAIL2_DELAY] = (jh // 2, 0)

            for idx, (h, jt, c0, w, e) in enumerate(steps):
                if jt == 0 and c0 == 0:
                    vts[h] = vtps.tile([DK + 1, LQ], F32, name="vt", tag="vt")
                pes[idx] = issue_score(idx)
                j = idx - LAG
                if j >= 0:
                    issue_pv(j, vts[steps[j][0]], pes.pop(j))
                    after_pv(j, idx)
                t2 = tail2_at.pop(idx, None)
                if t2 is not None:
                    issue_tail2(*t2)
            for j in range(len(steps) - LAG, len(steps)):
                issue_pv(j, vts[steps[j][0]], pes.pop(j))
                after_pv(j, len(steps))
            for _, t2 in sorted(tail2_at.items()):
                issue_tail2(*t2)

            # ---- output projection (reuses PSUM slots so it can overlap
            # the last head pair's tail) ----
            # i-tiles 4..7 first: they read only the high vtn columns,
            # whose normalize finished mid-head-7, so they overlap the
            # last pair's low-half tail chain
            with tc.tile_pool(name="obuf", bufs=3) as obuf:
                for it in list(range(4, NIT)) + list(range(4)):
                    po = stps.tile([P, D], F32, name="po", tag="st")
                    for c in range(DC):
                        nc.tensor.matmul(
                            po, lhsT=vtn_sb[:, c, it * P:(it + 1) * P],
                            rhs=wo_sb[:, c, :], start=(c == 0),
                            stop=(c == DC - 1))
                    ob = obuf.tile([P, D], F16, tag="ob")
                    nc.vector.tensor_add(ob, po, bo_bc)
                    nc.sync.dma_start(out=out[it * P:(it + 1) * P, :], in_=ob)


_NC_CACHE = None


def _get_nc():
    global _NC_CACHE
    if _NC_CACHE is None:
        _NC_CACHE = build_nc()
    return _NC_CACHE


def _sel65_const():
    # 16x compensates the 1/16 scale on the unnormalized-VT f16 stash
    sel = np.zeros((DK + 1, P), dtype=np.float16)
    sel[0, 0:DK] = 16.0
    sel[DK, DK:P] = 16.0
    return sel


def make_in_maps(x_q, x_k, x_v, padding_mask, attention_mask,
                 Wq, bq, Wk, bk, Wv, bv, Wo, bo):
    f16, f32 = np.float16, np.float32
    shared = {
        "wqT": np.ascontiguousarray(np.asarray(Wq, dtype=f32).T).astype(f16),
        "wkT": np.ascontiguousarray(np.asarray(Wk, dtype=f32).T).astype(f16),
        "wvT": np.ascontiguousarray(np.asarray(Wv, dtype=f32).T).astype(f16),
        "woT": np.ascontiguousarray(np.asarray(Wo, dtype=f32).T).astype(f16),
        "bq": np.asarray(bq, dtype=f32), "bk": np.asarray(bk, dtype=f32),
        "bv": np.asarray(bv, dtype=f32), "bo": np.asarray(bo, dtype=f32),
        "sel65d": _sel65_const(),
        "identd": np.eye(P, dtype=f16),
    }
    am = np.asarray(attention_mask, dtype=f32)
    # additive edge-mask bias [i, jt, j] (0 keep / -400 drop) for the
    # last local i-tile of each jt prefix
    edge_half = []
    for half in range(2):
        loc = LOCAL_TILES[half]
        e = np.empty((NJT, P, P), dtype=f16)       # [jt, i, j]
        for jt in range(NJT):
            g = loc[PROFILE[jt] - 1]
            blk = am[g * P:(g + 1) * P, jt * P:(jt + 1) * P]
            e[jt] = ((blk - 1.0) * 400.0).astype(f16)
        edge_half.append(np.ascontiguousarray(e.transpose(1, 0, 2)))
    xT = [np.asarray(x, dtype=f32).transpose(0, 2, 1).astype(f16)
          for x in (x_q, x_k, x_v)]
    in_maps = []
    for core in range(NCORES):
        n, half = divmod(core, 2)
        loc = LOCAL_TILES[half]
        xq_loc = xT[0][n].reshape(D, NJT, P)[:, loc, :].reshape(D, LQ)
        padb = (np.asarray(padding_mask[n], dtype=f32) - 1.0) * 1e9
        in_maps.append(dict(
            shared,
            xqT=np.ascontiguousarray(xq_loc),
            xkT=np.ascontiguousarray(xT[1][n]),
            xvT=np.ascontiguousarray(xT[2][n]),
            edgeb=edge_half[half],
            padb=padb,
        ))
    return in_maps


def gather_out(results):
    full = np.empty((N, L, D), dtype=np.float32)
    for core in range(NCORES):
        n, half = divmod(core, 2)
        o = results[core]["out"]
        for l, g in enumerate(LOCAL_TILES[half]):
            full[n, g * P:(g + 1) * P, :] = o[l * P:(l + 1) * P, :]
    return full


def kernel(x_q, x_k, x_v, padding_mask, attention_mask,
           Wq, bq, Wk, bk, Wv, bv, Wo, bo):
    nc = _get_nc()
    in_maps = make_in_maps(x_q, x_k, x_v, padding_mask, attention_mask,
                           Wq, bq, Wk, bk, Wv, bv, Wo, bo)
    res = run_bass_kernel_spmd(nc, in_maps, core_ids=list(range(NCORES)))
    return gather_out(res.results)
